# revision 20
# baseline (speedup 1.0000x reference)
"""NavierStokesSplittingEuler trn2 kernel, 8-core SPMD — single-shot
folded-operator design.

Sharding: x-axis 4-way per channel (core c: channel c//4, x-rows
[128*(c%4), 128*(c%4)+128)).  The 1000 Jacobi iterations are evaluated
in CLOSED FORM: phi_1000 = C_S * P - C_Q * B'  where C_S = IDFT(s^1000)
and C_Q = IDFT((1-s^1000)/(1-s)) are 512-periodic circulant kernels
(s = Jacobi symbol), both strongly separable (SVD rank 4 and 12).
Each apply is two stages of PE matmuls: stage 1 contracts the x-axis
against per-rank circulant profiles with the field as the stationary
operand (fusing the transpose); stage 2 contracts y against circulant
profile slices, accumulating all ranks into one PSUM tile (fusing the
transpose back).  Cross-core communication is two AllGathers total:
one to share B' (overlapped with the S-apply) and a 2-row one for the
final pressure gradient.
"""
import os
import sys

for _p in ("/opt/trn_rl_repo", "/root/.axon_site/_ro/trn_rl_repo"):
    if os.path.isdir(_p) and _p not in sys.path:
        sys.path.append(_p)

import numpy as np
import concourse.bass as bass
import concourse.tile as tile
from concourse import bacc, mybir
from concourse.bass import ds
from concourse.bass_utils import run_bass_kernel_spmd

F32 = mybir.dt.float32
N = 512
C = 2
NCORE = 8
NIT = 50 * int(os.environ.get("NSK_NPH", "20"))  # jacobi iterations
OWN = 128
TB = 122         # V_star work-tile rows (A/B tiles)
PADL = 52
PADR = 60
BOFF = 118       # B-tile row offset in the input block
BLK = OWN + PADL + PADR  # 240 input rows per core
RS1 = 4          # rank of folded S^NIT kernel
FP32R = os.environ.get("NSK_FP32R", "0") == "1"
RQ1 = 12         # rank of folded Q kernel

DT, BETA, RHO, NU = 0.1, 0.5, 1.0, 0.1
CADV = -DT
CLAPC = 1.0 - 4.0 * DT * NU
CLAPN = DT * NU
CGP = -DT * BETA / RHO / 2.0    # coeff on raw (P[+1]-P[-1]) diffs
CBD = RHO / (4.0 * DT) / 2.0    # B' = CBD*(xdiff+ydiff)
CGF = -DT / RHO / 2.0           # V_new grad(Phi) coeff on raw diffs


def _band(nrows, ncols, entries):
    s = np.zeros((nrows, ncols), np.float32)
    for off, cf in entries.items():
        for m in range(ncols):
            kk = m + off
            if 0 <= kk < nrows:
                s[kk, m] += cf
    return s


def _fold_kernels(nit):
    kx = np.arange(N)
    c1 = np.cos(2 * np.pi * kx / N)
    s = (c1[:, None] + c1[None, :]) / 2.0
    sn = s**nit
    with np.errstate(divide='ignore', invalid='ignore'):
        q = np.where(np.abs(1 - s) < 1e-14, float(nit), (1 - sn) / (1 - s))
    CS = np.real(np.fft.ifft2(sn))
    CQ = np.real(np.fft.ifft2(q))
    return CS, CQ


def _sep_profiles(Ck, rank, neg=False):
    U, sv, Vt = np.linalg.svd(Ck)
    u = (U[:, :rank] * sv[:rank]).T
    v = Vt[:rank]
    if neg:
        v = -v
    return u, v


def _build_usf(u, rank):
    """Stage-1 moving constants, rotated frame: [128, 4*rank*128];
    slice xt: usf[:, xt*rank*128 : (xt+1)*rank*128]."""
    idx_p = np.arange(128)[:, None]
    idx_n = np.arange(128)[None, :]
    out = np.zeros((128, 4 * rank * 128), np.float32)
    for xt in range(4):
        for r in range(rank):
            blk = u[r][(128 * xt + idx_p - idx_n) % N]
            out[:, xt * rank * 128 + r * 128:
                xt * rank * 128 + (r + 1) * 128] = blk
    return out


def _build_vc(v, rank):
    """Stage-2 moving constants [128, rank*4*N]."""
    out = np.zeros((128, rank * 4 * N), np.float32)
    idx_p = np.arange(128)[:, None]
    idx_n = np.arange(N)[None, :]
    for r in range(rank):
        for tp in range(4):
            out[:, (r * 4 + tp) * N:(r * 4 + tp + 1) * N] = \
                v[r][(128 * tp + idx_p - idx_n) % N]
    return out


def _build_consts():
    CS, CQ = _fold_kernels(NIT)
    uS, vS = _sep_profiles(CS, RS1)
    uQ, vQ = _sep_profiles(CQ, RQ1, neg=True)

    sml = {}
    sml["lin"] = _band(TB, TB, {0: CLAPC, 1: CLAPN, -1: CLAPN})
    sml["eyelapn"] = (CLAPN * np.eye(TB)).astype(np.float32)
    sml["gx"] = _band(TB, TB, {1: 0.5, -1: -0.5})
    sml["gpx"] = _band(TB, TB, {1: CGP, -1: -CGP})
    sml["eyegp"] = (CGP * np.eye(TB)).astype(np.float32)
    sml["eyegpn"] = (-CGP * np.eye(TB)).astype(np.float32)
    sml["bdx"] = _band(TB, TB, {1: CBD, -1: -CBD})
    sml["gphi"] = _band(OWN, OWN, {1: CGF, -1: -CGF})
    gph2 = np.zeros((2, OWN), np.float32)
    gph2[0, 0] = -CGF        # phi[x0-1] term at out row 0
    gph2[1, OWN - 1] = CGF   # phi[x0+128] term at out row 127
    sml["gphih"] = gph2
    sela = np.zeros((TB, OWN), np.float32)
    for m in range(0, 69):
        sela[m + 52, m] = 1.0
    selb = np.zeros((TB, OWN), np.float32)
    for m in range(69, OWN):
        selb[m - 66, m] = 1.0
    sml["sela"] = sela
    sml["selb"] = selb

    consts = {"usfs": _build_usf(uS, RS1), "vcs": _build_vc(vS, RS1),
              "usfq": _build_usf(uQ, RQ1), "vcq": _build_vc(vQ, RQ1)}
    consts.update(sml)
    return consts


_PROG = None


def _build_program(consts):
    nc = bacc.Bacc("TRN2", target_bir_lowering=False, debug=False,
                   enable_asserts=True, num_devices=NCORE)
    vblk = nc.declare_dram_parameter("vblk", [2, BLK, N], F32, isOutput=False)
    pblk = nc.declare_dram_parameter("pblk", [BLK, N], F32, isOutput=False)
    pfull = nc.declare_dram_parameter("pfull", [N, N], F32, isOutput=False)
    dram_in = {k: nc.declare_dram_parameter(k, list(v.shape), F32,
                                            isOutput=False)
               for k, v in consts.items()}
    vout = nc.declare_dram_parameter("vout", [2, OWN, N], F32, isOutput=True)
    pout = nc.declare_dram_parameter("pout", [OWN, N], F32, isOutput=True)

    groups = [[0, 1, 2, 3], [4, 5, 6, 7]]
    AOP = mybir.AluOpType
    SMALL = ("lin", "eyelapn", "gx", "gpx", "eyegp", "eyegpn", "bdx",
             "gphi", "gphih", "sela", "selb")
    QW = RQ1 * 128   # 1536 = 3 groups of 512

    with tile.TileContext(nc) as tc:
        with tc.tile_pool(name="const", bufs=1) as cpool, \
             tc.tile_pool(name="data", bufs=1) as dpool, \
             tc.tile_pool(name="qstream", bufs=6) as qpool, \
             tc.tile_pool(name="scratch", bufs=1) as spool, \
             tc.tile_pool(name="psA", bufs=2, space="PSUM") as psa, \
             tc.tile_pool(name="psB", bufs=2, space="PSUM") as psb, \
             tc.tile_pool(name="dram", bufs=1, space="DRAM") as drpool:

            # ---- field inputs (sync queue, needed first) ----
            va = dpool.tile([TB, N + 2], F32, tag="va0")
            vb = dpool.tile([TB, N + 2], F32, tag="vb0")
            wa = dpool.tile([TB, N + 2], F32, tag="wa1")
            wb = dpool.tile([TB, N + 2], F32, tag="wb1")
            pa = dpool.tile([TB, N + 2], F32, tag="pa")
            pb = dpool.tile([TB, N + 2], F32, tag="pb")
            for t, src, r0 in ((va, 0, 0), (vb, 0, BOFF), (wa, 1, 0),
                               (wb, 1, BOFF)):
                nc.sync.dma_start(t[:, 1:N + 1], vblk[src, r0:r0 + TB, :])
                nc.sync.dma_start(t[:, 0:1], vblk[src, r0:r0 + TB, N - 1:N])
                nc.sync.dma_start(t[:, N + 1:N + 2], vblk[src, r0:r0 + TB, 0:1])
            for t, r0 in ((pa, 0), (pb, BOFF)):
                nc.sync.dma_start(t[:, 1:N + 1], pblk[r0:r0 + TB, :])
                nc.sync.dma_start(t[:, 0:1], pblk[r0:r0 + TB, N - 1:N])
                nc.sync.dma_start(t[:, N + 1:N + 2], pblk[r0:r0 + TB, 0:1])
            pown = dpool.tile([OWN, N], F32, tag="pown")
            nc.sync.dma_start(pown[:], pblk[PADL:PADL + OWN, :])
            pf = []
            for xt in range(4):
                t = dpool.tile([128, N], F32, tag=f"pf{xt}", name=f"pf{xt}")
                nc.sync.dma_start(t[:], pfull[128 * xt:128 * xt + 128, :])
                pf.append(t)

            # ---- constants (small on sync, large streamed on vector queue) ----
            smt = {}
            for k in SMALL:
                smt[k] = cpool.tile(list(consts[k].shape), F32, tag=f"sm_{k}",
                                    name=f"sm_{k}")
                nc.sync.dma_start(smt[k][:], dram_in[k][:])
            usfs = cpool.tile([128, 4 * RS1 * 128], F32, tag="usfs")
            nc.scalar.dma_start(usfs[:], dram_in["usfs"][:])
            vcs = cpool.tile([128, RS1 * 4 * N], F32, tag="vcs")
            nc.scalar.dma_start(vcs[:], dram_in["vcs"][:])
            usfq = cpool.tile([128, 4 * QW], F32, tag="usfq")
            nc.scalar.dma_start(usfq[:], dram_in["usfq"][:])

            # ---- V_star ----
            vstar = {}
            for comp in (0, 1):
                for half in ("a", "b"):
                    vt = (va, vb)[half == "b"] if comp == 0 else (wa, wb)[half == "b"]
                    v0t = (va, vb)[half == "b"]
                    v1t = (wa, wb)[half == "b"]
                    pt = (pa, pb)[half == "b"]
                    ps_lin = psa.tile([TB, N], F32, tag="pslin")
                    nc.tensor.matmul(ps_lin[:], smt["lin"][:], vt[:, 1:N + 1],
                                     start=True, stop=False)
                    nc.tensor.matmul(ps_lin[:], smt["eyelapn"][:], vt[:, 0:N],
                                     start=False, stop=False)
                    nc.tensor.matmul(ps_lin[:], smt["eyelapn"][:], vt[:, 2:N + 2],
                                     start=False, stop=False)
                    if comp == 0:
                        nc.tensor.matmul(ps_lin[:], smt["gpx"][:], pt[:, 1:N + 1],
                                         start=False, stop=True)
                    else:
                        nc.tensor.matmul(ps_lin[:], smt["eyegp"][:], pt[:, 2:N + 2],
                                         start=False, stop=False)
                        nc.tensor.matmul(ps_lin[:], smt["eyegpn"][:], pt[:, 0:N],
                                         start=False, stop=True)
                    ps_dx = psb.tile([TB, N], F32, tag="psdx")
                    nc.tensor.matmul(ps_dx[:], smt["gx"][:], vt[:, 1:N + 1],
                                     start=True, stop=True)
                    yd = spool.tile([TB, N], F32, tag="yd")
                    nc.vector.tensor_sub(yd[:], vt[:, 2:N + 2], vt[:, 0:N])
                    m2 = spool.tile([TB, N], F32, tag="m2")
                    nc.gpsimd.tensor_mul(m2[:], v1t[:, 1:N + 1], yd[:])
                    m1 = spool.tile([TB, N], F32, tag="m1")
                    nc.vector.tensor_mul(m1[:], v0t[:, 1:N + 1], ps_dx[:])
                    adv = spool.tile([TB, N], F32, tag="adv")
                    nc.vector.scalar_tensor_tensor(adv[:], m2[:], 0.5, m1[:],
                                                   AOP.mult, AOP.add)
                    vs = dpool.tile([TB, N], F32, tag=f"vs{comp}{half}")
                    nc.vector.scalar_tensor_tensor(vs[:], adv[:], CADV, ps_lin[:],
                                                   AOP.mult, AOP.add)
                    vstar[(comp, half)] = vs

            # ---- B' on owned rows ----
            bpo = dpool.tile([OWN, N], F32, tag="bpo")
            for half in ("a", "b"):
                ps_b = psb.tile([TB, N], F32, tag="psdx")
                nc.tensor.matmul(ps_b[:], smt["bdx"][:], vstar[(0, half)][:],
                                 start=True, stop=True)
                vs1 = vstar[(1, half)]
                ydb = spool.tile([TB, N], F32, tag="yd")
                nc.vector.tensor_sub(ydb[:, 1:N - 1], vs1[:, 2:N], vs1[:, 0:N - 2])
                nc.vector.tensor_sub(ydb[:, 0:1], vs1[:, 1:2], vs1[:, N - 1:N])
                nc.vector.tensor_sub(ydb[:, N - 1:N], vs1[:, 0:1],
                                     vs1[:, N - 2:N - 1])
                bp = spool.tile([TB, N], F32, tag=f"bp{half}")
                nc.vector.scalar_tensor_tensor(bp[:], ydb[:], CBD, ps_b[:],
                                               AOP.mult, AOP.add)
                if half == "a":
                    nc.sync.dma_start(bpo[0:68, :], bp[52:120, :])
                else:
                    nc.sync.dma_start(bpo[68:OWN, :], bp[2:62, :])

            # ---- share B' across the x-ring (overlapped with S-apply) ----
            agB = drpool.tile([OWN, N], F32, tag="agB")
            agoB = drpool.tile([N, N], F32, tag="agoB")
            nc.sync.dma_start(agB[:], bpo[:])
            nc.gpsimd.collective_compute(
                "AllGather", AOP.bypass, replica_groups=groups,
                ins=[agB[:]], outs=[agoB[:]])
            pid = nc.sync.partition_id()
            bf = []
            for xt in range(4):
                t = dpool.tile([128, N], F32, tag=f"bf{xt}", name=f"bf{xt}")
                off = ((pid + xt) % 4) * 128
                nc.sync.dma_start(t[:], agoB[ds(off, 128), :])
                bf.append(t)

            def rr(ap):
                return ap.bitcast(mybir.dt.float32r) if FP32R else ap

            psF = psa.tile([OWN, N], F32, tag="psF", name="psF")
            nmm_f = RS1 * 4 + RQ1 * 4
            imm = 0

            # ---- S-apply on P (no cross-core dependency) ----
            atS = []
            for tp in range(4):
                psg = psb.tile([128, RS1 * 128], F32, tag="ps1",
                               name=f"psgS{tp}")
                for xt in range(4):
                    nc.tensor.matmul(psg[:],
                                     rr(pf[xt][:, 128 * tp:128 * tp + 128]),
                                     rr(usfs[:, xt * RS1 * 128:(xt + 1) * RS1 * 128]),
                                     start=(xt == 0), stop=(xt == 3))
                at = spool.tile([128, RS1 * 128], F32, tag="atS",
                                name=f"atS{tp}", bufs=4)
                nc.scalar.copy(at[:], psg[:])
                atS.append(at)
            for r in range(RS1):
                for tp in range(4):
                    nc.tensor.matmul(psF[:],
                                     rr(atS[tp][:, r * 128:(r + 1) * 128]),
                                     rr(vcs[:, (r * 4 + tp) * N:(r * 4 + tp + 1) * N]),
                                     start=(imm == 0), stop=(imm == nmm_f - 1))
                    imm += 1

            # ---- Q-apply on B' (negated; waits on the AllGather) ----
            atQ = []
            for tp in range(4):
                at = spool.tile([128, QW], F32, tag="atQ", name=f"atQ{tp}",
                                bufs=4)
                for g in range(3):
                    c0 = g * 512
                    psg = psb.tile([128, 512], F32, tag="ps1",
                                   name=f"psgQ{tp}_{g}")
                    for xt in range(4):
                        nc.tensor.matmul(
                            psg[:], rr(bf[xt][:, 128 * tp:128 * tp + 128]),
                            rr(usfq[:, xt * QW + c0:xt * QW + c0 + 512]),
                            start=(xt == 0), stop=(xt == 3))
                    nc.scalar.copy(at[:, c0:c0 + 512], psg[:])
                atQ.append(at)
            for r in range(RQ1):
                for tp in range(4):
                    mv = qpool.tile([128, N], F32, tag="qmov",
                                    name=f"qmov{r}_{tp}")
                    nc.sync.dma_start(
                        mv[:],
                        dram_in["vcq"][:, (r * 4 + tp) * N:(r * 4 + tp + 1) * N])
                    nc.tensor.matmul(psF[:],
                                     rr(atQ[tp][:, r * 128:(r + 1) * 128]),
                                     rr(mv[:]), start=(imm == 0),
                                     stop=(imm == nmm_f - 1))
                    imm += 1

            # ---- evict phi, then 2-row boundary share for grad(Phi) ----
            phiow = dpool.tile([OWN, N], F32, tag="phiow")
            nc.scalar.copy(phiow[0:32, :], psF[0:32, :])
            nc.scalar.copy(phiow[96:OWN, :], psF[96:OWN, :])
            agi2 = drpool.tile([2, N], F32, tag="agi2")
            ago2 = drpool.tile([8, N], F32, tag="ago2")
            nc.sync.dma_start(agi2[0:1, :], phiow[0:1, :])
            nc.sync.dma_start(agi2[1:2, :], phiow[OWN - 1:OWN, :])
            nc.gpsimd.collective_compute(
                "AllGather", AOP.bypass, replica_groups=groups,
                ins=[agi2[:]], outs=[ago2[:]])
            nc.scalar.copy(phiow[32:64, :], psF[32:64, :])
            nc.scalar.copy(phiow[64:96, :], psF[64:96, :])
            phih2 = dpool.tile([2, N], F32, tag="phih2")
            off_up = ((pid + 3) % 4) * 2 + 1
            off_dn = ((pid + 1) % 4) * 2
            nc.sync.dma_start(phih2[0:1, :], ago2[ds(off_up, 1), :])
            nc.sync.dma_start(phih2[1:2, :], ago2[ds(off_dn, 1), :])

            # ---- outputs (phih2-independent work first, to cover the AG) ----
            pn = spool.tile([OWN, N], F32, tag="pn")
            nc.vector.scalar_tensor_tensor(pn[:], pown[:], BETA, phiow[:],
                                           AOP.mult, AOP.add)
            nc.sync.dma_start(pout[:], pn[:])

            ps1 = psb.tile([OWN, N], F32, tag="psdx")
            nc.tensor.matmul(ps1[:], smt["sela"][:], vstar[(1, "a")][:],
                             start=True, stop=False)
            nc.tensor.matmul(ps1[:], smt["selb"][:], vstar[(1, "b")][:],
                             start=False, stop=True)
            ydp = spool.tile([OWN, N], F32, tag="ydp")
            nc.vector.tensor_sub(ydp[:, 1:N - 1], phiow[:, 2:N],
                                 phiow[:, 0:N - 2])
            nc.vector.tensor_sub(ydp[:, 0:1], phiow[:, 1:2], phiow[:, N - 1:N])
            nc.vector.tensor_sub(ydp[:, N - 1:N], phiow[:, 0:1],
                                 phiow[:, N - 2:N - 1])
            vn1 = spool.tile([OWN, N], F32, tag="vn1")
            nc.vector.scalar_tensor_tensor(vn1[:], ydp[:], CGF, ps1[:],
                                           AOP.mult, AOP.add)
            nc.sync.dma_start(vout[1], vn1[:])

            ps0 = psa.tile([OWN, N], F32, tag="pslin")
            nc.tensor.matmul(ps0[:], smt["sela"][:], vstar[(0, "a")][:],
                             start=True, stop=False)
            nc.tensor.matmul(ps0[:], smt["selb"][:], vstar[(0, "b")][:],
                             start=False, stop=False)
            nc.tensor.matmul(ps0[:], smt["gphi"][:], phiow[:],
                             start=False, stop=False)
            nc.tensor.matmul(ps0[:], smt["gphih"][:], phih2[:],
                             start=False, stop=True)
            vn0 = spool.tile([OWN, N], F32, tag="vn0")
            nc.scalar.copy(vn0[:], ps0[:])
            nc.sync.dma_start(vout[0], vn0[:])

    nc.finalize()
    return nc


def kernel(V, P):
    global _PROG
    V = np.ascontiguousarray(V, np.float32)
    P = np.ascontiguousarray(P, np.float32)
    if _PROG is None:
        consts = _build_consts()
        nc = _build_program(consts)
        _PROG = (nc, consts)
    nc, consts = _PROG
    in_maps = []
    for c in range(NCORE):
        ch, xb = c // 4, c % 4
        x0 = OWN * xb
        rows = np.arange(x0 - PADL, x0 + OWN + PADR) % N
        m = {"vblk": np.ascontiguousarray(V[:, ch][:, rows, :]),
             "pblk": np.ascontiguousarray(P[ch][rows, :]),
             "pfull": np.ascontiguousarray(np.roll(P[ch], -x0, axis=0))}
        m.update(consts)
        in_maps.append(m)
    trace = os.environ.get("NSK_TRACE", "") == "1"
    res = run_bass_kernel_spmd(nc, in_maps, core_ids=list(range(NCORE)),
                               trace=trace)
    if trace:
        print(f"HW exec time: {res.exec_time_ns} ns")
        if res.instructions_and_trace:
            print("trace:", res.instructions_and_trace[1])
    V_new = np.empty((2, C, N, N), np.float32)
    P_new = np.empty((C, N, N), np.float32)
    for c in range(NCORE):
        ch, xb = c // 4, c % 4
        x0 = OWN * xb
        V_new[:, ch, x0:x0 + OWN, :] = res.results[c]["vout"]
        P_new[ch, x0:x0 + OWN, :] = res.results[c]["pout"]
    return V_new, P_new


# revision 22
# speedup vs baseline: 1.1476x; 1.1476x over previous
"""NavierStokesSplittingEuler trn2 kernel, 8-core SPMD — single-shot
folded-operator design.

Sharding: x-axis 4-way per channel (core c: channel c//4, x-rows
[128*(c%4), 128*(c%4)+128)).  The 1000 Jacobi iterations are evaluated
in CLOSED FORM: phi_1000 = C_S * P - C_Q * B'  where C_S = IDFT(s^1000)
and C_Q = IDFT((1-s^1000)/(1-s)) are 512-periodic circulant kernels
(s = Jacobi symbol), both strongly separable (SVD rank 4 and 12).
Each apply is two stages of PE matmuls: stage 1 contracts the x-axis
against per-rank circulant profiles with the field as the stationary
operand (fusing the transpose); stage 2 contracts y against circulant
profile slices, accumulating all ranks into one PSUM tile (fusing the
transpose back).  Cross-core communication is two AllGathers total:
one to share B' (overlapped with the S-apply) and a 2-row one for the
final pressure gradient.
"""
import os
import sys

for _p in ("/opt/trn_rl_repo", "/root/.axon_site/_ro/trn_rl_repo"):
    if os.path.isdir(_p) and _p not in sys.path:
        sys.path.append(_p)

import numpy as np
import concourse.bass as bass
import concourse.tile as tile
from concourse import bacc, mybir
from concourse.bass import ds
from concourse.bass_utils import run_bass_kernel_spmd

F32 = mybir.dt.float32
N = 512
C = 2
NCORE = 8
NIT = 50 * int(os.environ.get("NSK_NPH", "20"))  # jacobi iterations
OWN = 128
TB = 69          # V_star work-tile rows (A/B tiles)
PADL = 3
PADR = 6
BOFF = 65        # B-tile row offset in the input block
BLK = OWN + PADL + PADR  # 136 input rows per core
RS1 = 4          # rank of folded S^NIT kernel
FP32R = os.environ.get("NSK_FP32R", "0") == "1"
RQ1 = 12         # rank of folded Q kernel

DT, BETA, RHO, NU = 0.1, 0.5, 1.0, 0.1
CADV = -DT
CLAPC = 1.0 - 4.0 * DT * NU
CLAPN = DT * NU
CGP = -DT * BETA / RHO / 2.0    # coeff on raw (P[+1]-P[-1]) diffs
CBD = RHO / (4.0 * DT) / 2.0    # B' = CBD*(xdiff+ydiff)
CGF = -DT / RHO / 2.0           # V_new grad(Phi) coeff on raw diffs


def _band(nrows, ncols, entries):
    s = np.zeros((nrows, ncols), np.float32)
    for off, cf in entries.items():
        for m in range(ncols):
            kk = m + off
            if 0 <= kk < nrows:
                s[kk, m] += cf
    return s


def _fold_kernels(nit):
    kx = np.arange(N)
    c1 = np.cos(2 * np.pi * kx / N)
    s = (c1[:, None] + c1[None, :]) / 2.0
    sn = s**nit
    with np.errstate(divide='ignore', invalid='ignore'):
        q = np.where(np.abs(1 - s) < 1e-14, float(nit), (1 - sn) / (1 - s))
    CS = np.real(np.fft.ifft2(sn))
    CQ = np.real(np.fft.ifft2(q))
    return CS, CQ


def _sep_profiles(Ck, rank, neg=False):
    U, sv, Vt = np.linalg.svd(Ck)
    u = (U[:, :rank] * sv[:rank]).T
    v = Vt[:rank]
    if neg:
        v = -v
    return u, v


def _build_usf(u, rank):
    """Stage-1 moving constants, rotated frame: [128, 4*rank*128];
    slice xt: usf[:, xt*rank*128 : (xt+1)*rank*128]."""
    idx_p = np.arange(128)[:, None]
    idx_n = np.arange(128)[None, :]
    out = np.zeros((128, 4 * rank * 128), np.float32)
    for xt in range(4):
        for r in range(rank):
            blk = u[r][(128 * xt + idx_p - idx_n) % N]
            out[:, xt * rank * 128 + r * 128:
                xt * rank * 128 + (r + 1) * 128] = blk
    return out


def _build_vc(v, rank):
    """Stage-2 moving constants [128, rank*4*N]."""
    out = np.zeros((128, rank * 4 * N), np.float32)
    idx_p = np.arange(128)[:, None]
    idx_n = np.arange(N)[None, :]
    for r in range(rank):
        for tp in range(4):
            out[:, (r * 4 + tp) * N:(r * 4 + tp + 1) * N] = \
                v[r][(128 * tp + idx_p - idx_n) % N]
    return out


def _build_consts():
    CS, CQ = _fold_kernels(NIT)
    uS, vS = _sep_profiles(CS, RS1)
    uQ, vQ = _sep_profiles(CQ, RQ1, neg=True)

    sml = {}
    sml["lin"] = _band(TB, TB, {0: CLAPC, 1: CLAPN, -1: CLAPN})
    sml["eyelapn"] = (CLAPN * np.eye(TB)).astype(np.float32)
    sml["gx"] = _band(TB, TB, {1: 0.5, -1: -0.5})
    sml["gpx"] = _band(TB, TB, {1: CGP, -1: -CGP})
    sml["eyegp"] = (CGP * np.eye(TB)).astype(np.float32)
    sml["eyegpn"] = (-CGP * np.eye(TB)).astype(np.float32)
    sml["bdx"] = _band(TB, TB, {1: CBD, -1: -CBD})
    sml["gphi"] = _band(OWN, OWN, {1: CGF, -1: -CGF})
    gph2 = np.zeros((2, OWN), np.float32)
    gph2[0, 0] = -CGF        # phi[x0-1] term at out row 0
    gph2[1, OWN - 1] = CGF   # phi[x0+128] term at out row 127
    sml["gphih"] = gph2
    sela = np.zeros((TB, OWN), np.float32)
    for m in range(0, 64):
        sela[m + PADL, m] = 1.0
    selb = np.zeros((TB, OWN), np.float32)
    for m in range(64, OWN):
        selb[m - 62, m] = 1.0
    sml["sela"] = sela
    sml["selb"] = selb

    consts = {"usfs": _build_usf(uS, RS1), "vcs": _build_vc(vS, RS1),
              "usfq": _build_usf(uQ, RQ1), "vcq": _build_vc(vQ, RQ1)}
    consts.update(sml)
    return consts


_PROG = None


def _build_program(consts):
    nc = bacc.Bacc("TRN2", target_bir_lowering=False, debug=False,
                   enable_asserts=True, num_devices=NCORE)
    vblk = nc.declare_dram_parameter("vblk", [2, BLK, N], F32, isOutput=False)
    pblk = nc.declare_dram_parameter("pblk", [BLK, N], F32, isOutput=False)
    pfull = nc.declare_dram_parameter("pfull", [N, N], F32, isOutput=False)
    dram_in = {k: nc.declare_dram_parameter(k, list(v.shape), F32,
                                            isOutput=False)
               for k, v in consts.items()}
    vout = nc.declare_dram_parameter("vout", [2, OWN, N], F32, isOutput=True)
    pout = nc.declare_dram_parameter("pout", [OWN, N], F32, isOutput=True)

    groups = [[0, 1, 2, 3], [4, 5, 6, 7]]
    AOP = mybir.AluOpType
    SMALL = ("lin", "eyelapn", "gx", "gpx", "eyegp", "eyegpn", "bdx",
             "gphi", "gphih", "sela", "selb")
    QW = RQ1 * 128   # 1536 = 3 groups of 512

    with tile.TileContext(nc) as tc:
        with tc.tile_pool(name="const", bufs=1) as cpool, \
             tc.tile_pool(name="data", bufs=1) as dpool, \
             tc.tile_pool(name="qstream", bufs=6) as qpool, \
             tc.tile_pool(name="scratch", bufs=1) as spool, \
             tc.tile_pool(name="psA", bufs=2, space="PSUM") as psa, \
             tc.tile_pool(name="psB", bufs=2, space="PSUM") as psb, \
             tc.tile_pool(name="dram", bufs=1, space="DRAM") as drpool:

            # ---- field inputs (sync queue, needed first) ----
            va = dpool.tile([TB, N + 2], F32, tag="va0")
            vb = dpool.tile([TB, N + 2], F32, tag="vb0")
            wa = dpool.tile([TB, N + 2], F32, tag="wa1")
            wb = dpool.tile([TB, N + 2], F32, tag="wb1")
            pa = dpool.tile([TB, N + 2], F32, tag="pa")
            pb = dpool.tile([TB, N + 2], F32, tag="pb")
            for t, eng, src, r0 in ((va, nc.sync, 0, 0), (vb, nc.scalar, 0, BOFF),
                                    (wa, nc.sync, 1, 0), (wb, nc.scalar, 1, BOFF)):
                eng.dma_start(t[:, 1:N + 1], vblk[src, r0:r0 + TB, :])
            for t, eng, r0 in ((pa, nc.sync, 0), (pb, nc.scalar, BOFF)):
                eng.dma_start(t[:, 1:N + 1], pblk[r0:r0 + TB, :])
            for t in (va, vb, wa, wb, pa, pb):
                nc.vector.tensor_copy(t[:, 0:1], t[:, N:N + 1])
                nc.vector.tensor_copy(t[:, N + 1:N + 2], t[:, 1:2])
            pown = dpool.tile([OWN, N], F32, tag="pown")
            nc.sync.dma_start(pown[:], pblk[PADL:PADL + OWN, :])
            pf = []
            for xt in range(4):
                t = dpool.tile([128, N], F32, tag=f"pf{xt}", name=f"pf{xt}")
                nc.gpsimd.dma_start(t[:], pfull[128 * xt:128 * xt + 128, :])
                pf.append(t)

            # ---- constants (small on sync, large streamed on vector queue) ----
            smt = {}
            for k in SMALL:
                smt[k] = cpool.tile(list(consts[k].shape), F32, tag=f"sm_{k}",
                                    name=f"sm_{k}")
                nc.sync.dma_start(smt[k][:], dram_in[k][:])
            usfs = cpool.tile([128, 4 * RS1 * 128], F32, tag="usfs")
            nc.scalar.dma_start(usfs[:], dram_in["usfs"][:])
            vcs = cpool.tile([128, RS1 * 4 * N], F32, tag="vcs")
            nc.scalar.dma_start(vcs[:], dram_in["vcs"][:])
            usfq = cpool.tile([128, 4 * QW], F32, tag="usfq")
            nc.scalar.dma_start(usfq[:], dram_in["usfq"][:])

            # ---- V_star ----
            vstar = {}
            for comp in (0, 1):
                for half in ("a", "b"):
                    vt = (va, vb)[half == "b"] if comp == 0 else (wa, wb)[half == "b"]
                    v0t = (va, vb)[half == "b"]
                    v1t = (wa, wb)[half == "b"]
                    pt = (pa, pb)[half == "b"]
                    ps_lin = psa.tile([TB, N], F32, tag="pslin")
                    nc.tensor.matmul(ps_lin[:], smt["lin"][:], vt[:, 1:N + 1],
                                     start=True, stop=False)
                    nc.tensor.matmul(ps_lin[:], smt["eyelapn"][:], vt[:, 0:N],
                                     start=False, stop=False)
                    nc.tensor.matmul(ps_lin[:], smt["eyelapn"][:], vt[:, 2:N + 2],
                                     start=False, stop=False)
                    if comp == 0:
                        nc.tensor.matmul(ps_lin[:], smt["gpx"][:], pt[:, 1:N + 1],
                                         start=False, stop=True)
                    else:
                        nc.tensor.matmul(ps_lin[:], smt["eyegp"][:], pt[:, 2:N + 2],
                                         start=False, stop=False)
                        nc.tensor.matmul(ps_lin[:], smt["eyegpn"][:], pt[:, 0:N],
                                         start=False, stop=True)
                    ps_dx = psb.tile([TB, N], F32, tag="psdx")
                    nc.tensor.matmul(ps_dx[:], smt["gx"][:], vt[:, 1:N + 1],
                                     start=True, stop=True)
                    yd = spool.tile([TB, N], F32, tag="yd")
                    nc.vector.tensor_sub(yd[:], vt[:, 2:N + 2], vt[:, 0:N])
                    m2 = spool.tile([TB, N], F32, tag="m2")
                    nc.gpsimd.tensor_mul(m2[:], v1t[:, 1:N + 1], yd[:])
                    m1 = spool.tile([TB, N], F32, tag="m1")
                    nc.vector.tensor_mul(m1[:], v0t[:, 1:N + 1], ps_dx[:])
                    adv = spool.tile([TB, N], F32, tag="adv")
                    nc.vector.scalar_tensor_tensor(adv[:], m2[:], 0.5, m1[:],
                                                   AOP.mult, AOP.add)
                    vs = dpool.tile([TB, N], F32, tag=f"vs{comp}{half}")
                    nc.vector.scalar_tensor_tensor(vs[:], adv[:], CADV, ps_lin[:],
                                                   AOP.mult, AOP.add)
                    vstar[(comp, half)] = vs

            # ---- B' on owned rows ----
            bpo = dpool.tile([OWN, N], F32, tag="bpo")
            for half in ("a", "b"):
                ps_b = psb.tile([TB, N], F32, tag="psdx")
                nc.tensor.matmul(ps_b[:], smt["bdx"][:], vstar[(0, half)][:],
                                 start=True, stop=True)
                vs1 = vstar[(1, half)]
                ydb = spool.tile([TB, N], F32, tag="yd")
                nc.vector.tensor_sub(ydb[:, 1:N - 1], vs1[:, 2:N], vs1[:, 0:N - 2])
                nc.vector.tensor_sub(ydb[:, 0:1], vs1[:, 1:2], vs1[:, N - 1:N])
                nc.vector.tensor_sub(ydb[:, N - 1:N], vs1[:, 0:1],
                                     vs1[:, N - 2:N - 1])
                bp = spool.tile([TB, N], F32, tag=f"bp{half}")
                nc.vector.scalar_tensor_tensor(bp[:], ydb[:], CBD, ps_b[:],
                                               AOP.mult, AOP.add)
                if half == "a":
                    nc.sync.dma_start(bpo[0:64, :], bp[3:67, :])
                else:
                    nc.sync.dma_start(bpo[64:OWN, :], bp[2:66, :])

            # ---- share B' across the x-ring (overlapped with S-apply) ----
            agB = drpool.tile([OWN, N], F32, tag="agB")
            agoB = drpool.tile([N, N], F32, tag="agoB")
            nc.sync.dma_start(agB[:], bpo[:])
            nc.gpsimd.collective_compute(
                "AllGather", AOP.bypass, replica_groups=groups,
                ins=[agB[:]], outs=[agoB[:]])
            pid = nc.sync.partition_id()
            bf = []
            for xt in range(4):
                t = dpool.tile([128, N], F32, tag=f"bf{xt}", name=f"bf{xt}")
                off = ((pid + xt) % 4) * 128
                nc.sync.dma_start(t[:], agoB[ds(off, 128), :])
                bf.append(t)

            def rr(ap):
                return ap.bitcast(mybir.dt.float32r) if FP32R else ap

            psF = psa.tile([OWN, N], F32, tag="psF", name="psF")
            nmm_f = RS1 * 4 + RQ1 * 4
            imm = 0

            # ---- S-apply on P (no cross-core dependency) ----
            atS = []
            for tp in range(4):
                psg = psb.tile([128, RS1 * 128], F32, tag="ps1",
                               name=f"psgS{tp}")
                for xt in range(4):
                    nc.tensor.matmul(psg[:],
                                     rr(pf[xt][:, 128 * tp:128 * tp + 128]),
                                     rr(usfs[:, xt * RS1 * 128:(xt + 1) * RS1 * 128]),
                                     start=(xt == 0), stop=(xt == 3))
                at = spool.tile([128, RS1 * 128], F32, tag="atS",
                                name=f"atS{tp}", bufs=4)
                nc.scalar.copy(at[:], psg[:])
                atS.append(at)
            for r in range(RS1):
                for tp in range(4):
                    nc.tensor.matmul(psF[:],
                                     rr(atS[tp][:, r * 128:(r + 1) * 128]),
                                     rr(vcs[:, (r * 4 + tp) * N:(r * 4 + tp + 1) * N]),
                                     start=(imm == 0), stop=(imm == nmm_f - 1))
                    imm += 1

            # ---- Q-apply on B' (negated; waits on the AllGather) ----
            atQ = []
            for tp in range(4):
                at = spool.tile([128, QW], F32, tag="atQ", name=f"atQ{tp}",
                                bufs=4)
                for g in range(3):
                    c0 = g * 512
                    psg = psb.tile([128, 512], F32, tag="ps1",
                                   name=f"psgQ{tp}_{g}")
                    for xt in range(4):
                        nc.tensor.matmul(
                            psg[:], rr(bf[xt][:, 128 * tp:128 * tp + 128]),
                            rr(usfq[:, xt * QW + c0:xt * QW + c0 + 512]),
                            start=(xt == 0), stop=(xt == 3))
                    nc.scalar.copy(at[:, c0:c0 + 512], psg[:])
                atQ.append(at)
            for r in range(RQ1):
                for tp in range(4):
                    mv = qpool.tile([128, N], F32, tag="qmov",
                                    name=f"qmov{r}_{tp}")
                    nc.sync.dma_start(
                        mv[:],
                        dram_in["vcq"][:, (r * 4 + tp) * N:(r * 4 + tp + 1) * N])
                    nc.tensor.matmul(psF[:],
                                     rr(atQ[tp][:, r * 128:(r + 1) * 128]),
                                     rr(mv[:]), start=(imm == 0),
                                     stop=(imm == nmm_f - 1))
                    imm += 1

            # ---- evict phi, then 2-row boundary share for grad(Phi) ----
            phiow = dpool.tile([OWN, N], F32, tag="phiow")
            nc.scalar.copy(phiow[0:32, :], psF[0:32, :])
            nc.scalar.copy(phiow[96:OWN, :], psF[96:OWN, :])
            agi2 = drpool.tile([2, N], F32, tag="agi2")
            ago2 = drpool.tile([8, N], F32, tag="ago2")
            nc.sync.dma_start(agi2[0:1, :], phiow[0:1, :])
            nc.sync.dma_start(agi2[1:2, :], phiow[OWN - 1:OWN, :])
            nc.gpsimd.collective_compute(
                "AllGather", AOP.bypass, replica_groups=groups,
                ins=[agi2[:]], outs=[ago2[:]])
            nc.scalar.copy(phiow[32:64, :], psF[32:64, :])
            nc.scalar.copy(phiow[64:96, :], psF[64:96, :])
            phih2 = dpool.tile([2, N], F32, tag="phih2")
            off_up = ((pid + 3) % 4) * 2 + 1
            off_dn = ((pid + 1) % 4) * 2
            nc.sync.dma_start(phih2[0:1, :], ago2[ds(off_up, 1), :])
            nc.sync.dma_start(phih2[1:2, :], ago2[ds(off_dn, 1), :])

            # ---- outputs (phih2-independent work first, to cover the AG) ----
            pn = spool.tile([OWN, N], F32, tag="pn")
            nc.vector.scalar_tensor_tensor(pn[:], pown[:], BETA, phiow[:],
                                           AOP.mult, AOP.add)
            nc.sync.dma_start(pout[:], pn[:])

            ps1 = psb.tile([OWN, N], F32, tag="psdx")
            nc.tensor.matmul(ps1[:], smt["sela"][:], vstar[(1, "a")][:],
                             start=True, stop=False)
            nc.tensor.matmul(ps1[:], smt["selb"][:], vstar[(1, "b")][:],
                             start=False, stop=True)
            ydp = spool.tile([OWN, N], F32, tag="ydp")
            nc.vector.tensor_sub(ydp[:, 1:N - 1], phiow[:, 2:N],
                                 phiow[:, 0:N - 2])
            nc.vector.tensor_sub(ydp[:, 0:1], phiow[:, 1:2], phiow[:, N - 1:N])
            nc.vector.tensor_sub(ydp[:, N - 1:N], phiow[:, 0:1],
                                 phiow[:, N - 2:N - 1])
            vn1 = spool.tile([OWN, N], F32, tag="vn1")
            nc.vector.scalar_tensor_tensor(vn1[:], ydp[:], CGF, ps1[:],
                                           AOP.mult, AOP.add)
            nc.sync.dma_start(vout[1], vn1[:])

            ps0 = psa.tile([OWN, N], F32, tag="pslin")
            nc.tensor.matmul(ps0[:], smt["sela"][:], vstar[(0, "a")][:],
                             start=True, stop=False)
            nc.tensor.matmul(ps0[:], smt["selb"][:], vstar[(0, "b")][:],
                             start=False, stop=False)
            nc.tensor.matmul(ps0[:], smt["gphi"][:], phiow[:],
                             start=False, stop=False)
            nc.tensor.matmul(ps0[:], smt["gphih"][:], phih2[:],
                             start=False, stop=True)
            vn0 = spool.tile([OWN, N], F32, tag="vn0")
            nc.scalar.copy(vn0[:], ps0[:])
            nc.sync.dma_start(vout[0], vn0[:])

    nc.finalize()
    return nc


def kernel(V, P):
    global _PROG
    V = np.ascontiguousarray(V, np.float32)
    P = np.ascontiguousarray(P, np.float32)
    if _PROG is None:
        consts = _build_consts()
        nc = _build_program(consts)
        _PROG = (nc, consts)
    nc, consts = _PROG
    in_maps = []
    for c in range(NCORE):
        ch, xb = c // 4, c % 4
        x0 = OWN * xb
        rows = np.arange(x0 - PADL, x0 + OWN + PADR) % N
        m = {"vblk": np.ascontiguousarray(V[:, ch][:, rows, :]),
             "pblk": np.ascontiguousarray(P[ch][rows, :]),
             "pfull": np.ascontiguousarray(np.roll(P[ch], -x0, axis=0))}
        m.update(consts)
        in_maps.append(m)
    trace = os.environ.get("NSK_TRACE", "") == "1"
    res = run_bass_kernel_spmd(nc, in_maps, core_ids=list(range(NCORE)),
                               trace=trace)
    if trace:
        print(f"HW exec time: {res.exec_time_ns} ns")
        if res.instructions_and_trace:
            print("trace:", res.instructions_and_trace[1])
    V_new = np.empty((2, C, N, N), np.float32)
    P_new = np.empty((C, N, N), np.float32)
    for c in range(NCORE):
        ch, xb = c // 4, c % 4
        x0 = OWN * xb
        V_new[:, ch, x0:x0 + OWN, :] = res.results[c]["vout"]
        P_new[ch, x0:x0 + OWN, :] = res.results[c]["pout"]
    return V_new, P_new


# revision 23
# speedup vs baseline: 1.3415x; 1.1689x over previous
"""NavierStokesSplittingEuler trn2 kernel, 8-core SPMD — single-shot
folded-operator design.

Sharding: x-axis 4-way per channel (core c: channel c//4, x-rows
[128*(c%4), 128*(c%4)+128)).  The 1000 Jacobi iterations are evaluated
in CLOSED FORM: phi_1000 = C_S * P - C_Q * B'  where C_S = IDFT(s^1000)
and C_Q = IDFT((1-s^1000)/(1-s)) are 512-periodic circulant kernels
(s = Jacobi symbol), both strongly separable (SVD rank 4 and 12).
Each apply is two stages of PE matmuls: stage 1 contracts the x-axis
against per-rank circulant profiles with the field as the stationary
operand (fusing the transpose); stage 2 contracts y against circulant
profile slices, accumulating all ranks into one PSUM tile (fusing the
transpose back).  Cross-core communication is two AllGathers total:
one to share B' (overlapped with the S-apply) and a 2-row one for the
final pressure gradient.
"""
import os
import sys

for _p in ("/opt/trn_rl_repo", "/root/.axon_site/_ro/trn_rl_repo"):
    if os.path.isdir(_p) and _p not in sys.path:
        sys.path.append(_p)

import numpy as np
import concourse.bass as bass
import concourse.tile as tile
from concourse import bacc, mybir
from concourse.bass import ds
from concourse.bass_utils import run_bass_kernel_spmd

F32 = mybir.dt.float32
F32R = mybir.dt.float32r
N = 512
C = 2
NCORE = 8
NIT = 50 * int(os.environ.get("NSK_NPH", "20"))  # jacobi iterations
OWN = 128
TB = 69          # V_star work-tile rows (A/B tiles)
PADL = 3
PADR = 6
BOFF = 65        # B-tile row offset in the input block
BLK = OWN + PADL + PADR  # 136 input rows per core
RS1 = 4          # rank of folded S^NIT kernel
FP32R = os.environ.get("NSK_FP32R", "0") == "1"
RQ1 = 12         # rank of folded Q kernel

DT, BETA, RHO, NU = 0.1, 0.5, 1.0, 0.1
CADV = -DT
CLAPC = 1.0 - 4.0 * DT * NU
CLAPN = DT * NU
CGP = -DT * BETA / RHO / 2.0    # coeff on raw (P[+1]-P[-1]) diffs
CBD = RHO / (4.0 * DT) / 2.0    # B' = CBD*(xdiff+ydiff)
CGF = -DT / RHO / 2.0           # V_new grad(Phi) coeff on raw diffs


def _band(nrows, ncols, entries):
    s = np.zeros((nrows, ncols), np.float32)
    for off, cf in entries.items():
        for m in range(ncols):
            kk = m + off
            if 0 <= kk < nrows:
                s[kk, m] += cf
    return s


def _fold_kernels(nit):
    kx = np.arange(N)
    c1 = np.cos(2 * np.pi * kx / N)
    s = (c1[:, None] + c1[None, :]) / 2.0
    sn = s**nit
    with np.errstate(divide='ignore', invalid='ignore'):
        q = np.where(np.abs(1 - s) < 1e-14, float(nit), (1 - sn) / (1 - s))
    CS = np.real(np.fft.ifft2(sn))
    CQ = np.real(np.fft.ifft2(q))
    return CS, CQ


def _sep_profiles(Ck, rank, neg=False):
    U, sv, Vt = np.linalg.svd(Ck)
    u = (U[:, :rank] * sv[:rank]).T
    v = Vt[:rank]
    if neg:
        v = -v
    return u, v


def _build_usf(u, rank):
    """Stage-1 moving constants, rotated frame: [128, 4*rank*128];
    slice xt: usf[:, xt*rank*128 : (xt+1)*rank*128]."""
    idx_p = np.arange(128)[:, None]
    idx_n = np.arange(128)[None, :]
    out = np.zeros((128, 4 * rank * 128), np.float32)
    for xt in range(4):
        for r in range(rank):
            blk = u[r][(128 * xt + idx_p - idx_n) % N]
            out[:, xt * rank * 128 + r * 128:
                xt * rank * 128 + (r + 1) * 128] = blk
    return out


def _build_vc(v, rank):
    """Stage-2 moving constants [128, rank*4*N]."""
    out = np.zeros((128, rank * 4 * N), np.float32)
    idx_p = np.arange(128)[:, None]
    idx_n = np.arange(N)[None, :]
    for r in range(rank):
        for tp in range(4):
            out[:, (r * 4 + tp) * N:(r * 4 + tp + 1) * N] = \
                v[r][(128 * tp + idx_p - idx_n) % N]
    return out


def _build_consts():
    CS, CQ = _fold_kernels(NIT)
    uS, vS = _sep_profiles(CS, RS1)
    uQ, vQ = _sep_profiles(CQ, RQ1, neg=True)

    sml = {}
    sml["lin"] = _band(TB, TB, {0: CLAPC, 1: CLAPN, -1: CLAPN})
    sml["eyelapn"] = (CLAPN * np.eye(TB)).astype(np.float32)
    sml["gx"] = _band(TB, TB, {1: 0.5, -1: -0.5})
    sml["gpx"] = _band(TB, TB, {1: CGP, -1: -CGP})
    sml["eyegp"] = (CGP * np.eye(TB)).astype(np.float32)
    sml["eyegpn"] = (-CGP * np.eye(TB)).astype(np.float32)
    sml["bdx"] = _band(TB, TB, {1: CBD, -1: -CBD})
    sml["gphi"] = _band(OWN, OWN, {1: CGF, -1: -CGF})
    gph2 = np.zeros((2, OWN), np.float32)
    gph2[0, 0] = -CGF        # phi[x0-1] term at out row 0
    gph2[1, OWN - 1] = CGF   # phi[x0+128] term at out row 127
    sml["gphih"] = gph2
    sela = np.zeros((TB, OWN), np.float32)
    for m in range(0, 64):
        sela[m + PADL, m] = 1.0
    selb = np.zeros((TB, OWN), np.float32)
    for m in range(64, OWN):
        selb[m - 62, m] = 1.0
    sml["sela"] = sela
    sml["selb"] = selb

    consts = {"usfs": _build_usf(uS, RS1), "vcs": _build_vc(vS, RS1),
              "usfq": _build_usf(uQ, RQ1), "vcq": _build_vc(vQ, RQ1)}
    consts.update(sml)
    return consts


_PROG = None


def _build_program(consts):
    nc = bacc.Bacc("TRN2", target_bir_lowering=False, debug=False,
                   enable_asserts=True, num_devices=NCORE)
    vblk = nc.declare_dram_parameter("vblk", [2, BLK, N], F32, isOutput=False)
    pblk = nc.declare_dram_parameter("pblk", [BLK, N], F32, isOutput=False)
    FR = F32R if FP32R else F32
    pfull = nc.declare_dram_parameter("pfull", [N, N], FR, isOutput=False)
    APPLY_CONSTS = ("usfs", "vcs", "usfq", "vcq")
    dram_in = {k: nc.declare_dram_parameter(
                    k, list(v.shape), FR if k in APPLY_CONSTS else F32,
                    isOutput=False)
               for k, v in consts.items()}
    vout = nc.declare_dram_parameter("vout", [2, OWN, N], F32, isOutput=True)
    pout = nc.declare_dram_parameter("pout", [OWN, N], F32, isOutput=True)

    groups = [[0, 1, 2, 3], [4, 5, 6, 7]]
    AOP = mybir.AluOpType
    SMALL = ("lin", "eyelapn", "gx", "gpx", "eyegp", "eyegpn", "bdx",
             "gphi", "gphih", "sela", "selb")
    QW = RQ1 * 128   # 1536 = 3 groups of 512

    with tile.TileContext(nc) as tc:
        with tc.tile_pool(name="const", bufs=1) as cpool, \
             tc.tile_pool(name="data", bufs=1) as dpool, \
             tc.tile_pool(name="qstream", bufs=6) as qpool, \
             tc.tile_pool(name="scratch", bufs=1) as spool, \
             tc.tile_pool(name="psA", bufs=2, space="PSUM") as psa, \
             tc.tile_pool(name="psB", bufs=2, space="PSUM") as psb, \
             tc.tile_pool(name="dram", bufs=1, space="DRAM") as drpool:

            # ---- field inputs (sync queue, needed first) ----
            va = dpool.tile([TB, N + 2], F32, tag="va0")
            vb = dpool.tile([TB, N + 2], F32, tag="vb0")
            wa = dpool.tile([TB, N + 2], F32, tag="wa1")
            wb = dpool.tile([TB, N + 2], F32, tag="wb1")
            pa = dpool.tile([TB, N + 2], F32, tag="pa")
            pb = dpool.tile([TB, N + 2], F32, tag="pb")
            for t, eng, src, r0 in ((va, nc.sync, 0, 0), (vb, nc.scalar, 0, BOFF),
                                    (wa, nc.sync, 1, 0), (wb, nc.scalar, 1, BOFF)):
                eng.dma_start(t[:, 1:N + 1], vblk[src, r0:r0 + TB, :])
            for t, eng, r0 in ((pa, nc.sync, 0), (pb, nc.scalar, BOFF)):
                eng.dma_start(t[:, 1:N + 1], pblk[r0:r0 + TB, :])
            for t in (va, vb, wa, wb, pa, pb):
                nc.vector.tensor_copy(t[:, 0:1], t[:, N:N + 1])
                nc.vector.tensor_copy(t[:, N + 1:N + 2], t[:, 1:2])
            pown = dpool.tile([OWN, N], F32, tag="pown")
            nc.sync.dma_start(pown[:], pblk[PADL:PADL + OWN, :])
            pf = []
            for xt in range(4):
                t = dpool.tile([128, N], FR, tag=f"pf{xt}", name=f"pf{xt}")
                nc.gpsimd.dma_start(t[:], pfull[128 * xt:128 * xt + 128, :])
                pf.append(t)

            # ---- constants (small on sync, large streamed on vector queue) ----
            smt = {}
            for k in SMALL:
                smt[k] = cpool.tile(list(consts[k].shape), F32, tag=f"sm_{k}",
                                    name=f"sm_{k}")
                nc.sync.dma_start(smt[k][:], dram_in[k][:])
            FR = F32R if FP32R else F32
            usfs = cpool.tile([128, 4 * RS1 * 128], FR, tag="usfs")
            nc.scalar.dma_start(usfs[:], dram_in["usfs"][:])
            vcs = cpool.tile([128, RS1 * 4 * N], FR, tag="vcs")
            nc.scalar.dma_start(vcs[:], dram_in["vcs"][:])
            usfq = cpool.tile([128, 4 * QW], FR, tag="usfq")
            nc.scalar.dma_start(usfq[:], dram_in["usfq"][:])

            # ---- V_star ----
            vstar = {}
            for comp in (0, 1):
                for half in ("a", "b"):
                    vt = (va, vb)[half == "b"] if comp == 0 else (wa, wb)[half == "b"]
                    v0t = (va, vb)[half == "b"]
                    v1t = (wa, wb)[half == "b"]
                    pt = (pa, pb)[half == "b"]
                    ps_lin = psa.tile([TB, N], F32, tag="pslin")
                    nc.tensor.matmul(ps_lin[:], smt["lin"][:], vt[:, 1:N + 1],
                                     start=True, stop=False)
                    nc.tensor.matmul(ps_lin[:], smt["eyelapn"][:], vt[:, 0:N],
                                     start=False, stop=False)
                    nc.tensor.matmul(ps_lin[:], smt["eyelapn"][:], vt[:, 2:N + 2],
                                     start=False, stop=False)
                    if comp == 0:
                        nc.tensor.matmul(ps_lin[:], smt["gpx"][:], pt[:, 1:N + 1],
                                         start=False, stop=True)
                    else:
                        nc.tensor.matmul(ps_lin[:], smt["eyegp"][:], pt[:, 2:N + 2],
                                         start=False, stop=False)
                        nc.tensor.matmul(ps_lin[:], smt["eyegpn"][:], pt[:, 0:N],
                                         start=False, stop=True)
                    ps_dx = psb.tile([TB, N], F32, tag="psdx")
                    nc.tensor.matmul(ps_dx[:], smt["gx"][:], vt[:, 1:N + 1],
                                     start=True, stop=True)
                    yd = spool.tile([TB, N], F32, tag="yd")
                    nc.vector.tensor_sub(yd[:], vt[:, 2:N + 2], vt[:, 0:N])
                    m2 = spool.tile([TB, N], F32, tag="m2")
                    nc.gpsimd.tensor_mul(m2[:], v1t[:, 1:N + 1], yd[:])
                    m1 = spool.tile([TB, N], F32, tag="m1")
                    nc.vector.tensor_mul(m1[:], v0t[:, 1:N + 1], ps_dx[:])
                    adv = spool.tile([TB, N], F32, tag="adv")
                    nc.vector.scalar_tensor_tensor(adv[:], m2[:], 0.5, m1[:],
                                                   AOP.mult, AOP.add)
                    vs = dpool.tile([TB, N], F32, tag=f"vs{comp}{half}")
                    nc.vector.scalar_tensor_tensor(vs[:], adv[:], CADV, ps_lin[:],
                                                   AOP.mult, AOP.add)
                    vstar[(comp, half)] = vs

            # ---- B' on owned rows ----
            bpo = dpool.tile([OWN, N], F32, tag="bpo")
            for half in ("a", "b"):
                ps_b = psb.tile([TB, N], F32, tag="psdx")
                nc.tensor.matmul(ps_b[:], smt["bdx"][:], vstar[(0, half)][:],
                                 start=True, stop=True)
                vs1 = vstar[(1, half)]
                ydb = spool.tile([TB, N], F32, tag="yd")
                nc.vector.tensor_sub(ydb[:, 1:N - 1], vs1[:, 2:N], vs1[:, 0:N - 2])
                nc.vector.tensor_sub(ydb[:, 0:1], vs1[:, 1:2], vs1[:, N - 1:N])
                nc.vector.tensor_sub(ydb[:, N - 1:N], vs1[:, 0:1],
                                     vs1[:, N - 2:N - 1])
                bp = spool.tile([TB, N], F32, tag=f"bp{half}")
                nc.vector.scalar_tensor_tensor(bp[:], ydb[:], CBD, ps_b[:],
                                               AOP.mult, AOP.add)
                if half == "a":
                    nc.sync.dma_start(bpo[0:64, :], bp[3:67, :])
                else:
                    nc.sync.dma_start(bpo[64:OWN, :], bp[2:66, :])

            # ---- share B' across the x-ring (overlapped with S-apply) ----
            agB = drpool.tile([OWN, N], FR, tag="agB")
            agoB = drpool.tile([N, N], FR, tag="agoB")
            nc.sync.dma_start(agB[:], bpo[:].bitcast(FR))
            nc.gpsimd.collective_compute(
                "AllGather", AOP.bypass, replica_groups=groups,
                ins=[agB[:]], outs=[agoB[:]])
            pid = nc.sync.partition_id()
            bf = []
            for xt in range(4):
                t = dpool.tile([128, N], FR, tag=f"bf{xt}", name=f"bf{xt}")
                off = ((pid + xt) % 4) * 128
                nc.sync.dma_start(t[:], agoB[ds(off, 128), :])
                bf.append(t)

            psF = psa.tile([OWN, N], F32, tag="psF", name="psF")
            nmm_f = RS1 * 4 + RQ1 * 4
            imm = 0

            # ---- S-apply on P (no cross-core dependency) ----
            atS = []
            for tp in range(4):
                psg = psb.tile([128, RS1 * 128], F32, tag="ps1",
                               name=f"psgS{tp}")
                for xt in range(4):
                    nc.tensor.matmul(psg[:],
                                     pf[xt][:, 128 * tp:128 * tp + 128],
                                     usfs[:, xt * RS1 * 128:(xt + 1) * RS1 * 128],
                                     start=(xt == 0), stop=(xt == 3))
                at = spool.tile([128, RS1 * 128], FR, tag="atS",
                                name=f"atS{tp}", bufs=4)
                nc.scalar.copy(at[:], psg[:])
                atS.append(at)
            for r in range(RS1):
                for tp in range(4):
                    nc.tensor.matmul(psF[:],
                                     atS[tp][:, r * 128:(r + 1) * 128],
                                     vcs[:, (r * 4 + tp) * N:(r * 4 + tp + 1) * N],
                                     start=(imm == 0), stop=(imm == nmm_f - 1))
                    imm += 1

            # ---- Q-apply on B' (negated; waits on the AllGather) ----
            atQ = []
            for tp in range(4):
                at = spool.tile([128, QW], FR, tag="atQ", name=f"atQ{tp}",
                                bufs=4)
                for g in range(3):
                    c0 = g * 512
                    psg = psb.tile([128, 512], F32, tag="ps1",
                                   name=f"psgQ{tp}_{g}")
                    for xt in range(4):
                        nc.tensor.matmul(
                            psg[:], bf[xt][:, 128 * tp:128 * tp + 128],
                            usfq[:, xt * QW + c0:xt * QW + c0 + 512],
                            start=(xt == 0), stop=(xt == 3))
                    nc.scalar.copy(at[:, c0:c0 + 512], psg[:])
                atQ.append(at)
            for r in range(RQ1):
                for tp in range(4):
                    mv = qpool.tile([128, N], FR, tag="qmov",
                                    name=f"qmov{r}_{tp}")
                    nc.sync.dma_start(
                        mv[:],
                        dram_in["vcq"][:, (r * 4 + tp) * N:(r * 4 + tp + 1) * N])
                    nc.tensor.matmul(psF[:],
                                     atQ[tp][:, r * 128:(r + 1) * 128],
                                     mv[:], start=(imm == 0),
                                     stop=(imm == nmm_f - 1))
                    imm += 1

            # ---- evict phi, then 2-row boundary share for grad(Phi) ----
            phiow = dpool.tile([OWN, N], F32, tag="phiow")
            nc.scalar.copy(phiow[0:32, :], psF[0:32, :])
            nc.scalar.copy(phiow[96:OWN, :], psF[96:OWN, :])
            agi2 = drpool.tile([2, N], F32, tag="agi2")
            ago2 = drpool.tile([8, N], F32, tag="ago2")
            nc.sync.dma_start(agi2[0:1, :], phiow[0:1, :])
            nc.sync.dma_start(agi2[1:2, :], phiow[OWN - 1:OWN, :])
            nc.gpsimd.collective_compute(
                "AllGather", AOP.bypass, replica_groups=groups,
                ins=[agi2[:]], outs=[ago2[:]])
            nc.scalar.copy(phiow[32:64, :], psF[32:64, :])
            nc.scalar.copy(phiow[64:96, :], psF[64:96, :])
            phih2 = dpool.tile([2, N], F32, tag="phih2")
            off_up = ((pid + 3) % 4) * 2 + 1
            off_dn = ((pid + 1) % 4) * 2
            nc.sync.dma_start(phih2[0:1, :], ago2[ds(off_up, 1), :])
            nc.sync.dma_start(phih2[1:2, :], ago2[ds(off_dn, 1), :])

            # ---- outputs (phih2-independent work first, to cover the AG) ----
            pn = spool.tile([OWN, N], F32, tag="pn")
            nc.vector.scalar_tensor_tensor(pn[:], pown[:], BETA, phiow[:],
                                           AOP.mult, AOP.add)
            nc.sync.dma_start(pout[:], pn[:])

            ps1 = psb.tile([OWN, N], F32, tag="psdx")
            nc.tensor.matmul(ps1[:], smt["sela"][:], vstar[(1, "a")][:],
                             start=True, stop=False)
            nc.tensor.matmul(ps1[:], smt["selb"][:], vstar[(1, "b")][:],
                             start=False, stop=True)
            ydp = spool.tile([OWN, N], F32, tag="ydp")
            nc.vector.tensor_sub(ydp[:, 1:N - 1], phiow[:, 2:N],
                                 phiow[:, 0:N - 2])
            nc.vector.tensor_sub(ydp[:, 0:1], phiow[:, 1:2], phiow[:, N - 1:N])
            nc.vector.tensor_sub(ydp[:, N - 1:N], phiow[:, 0:1],
                                 phiow[:, N - 2:N - 1])
            vn1 = spool.tile([OWN, N], F32, tag="vn1")
            nc.vector.scalar_tensor_tensor(vn1[:], ydp[:], CGF, ps1[:],
                                           AOP.mult, AOP.add)
            nc.sync.dma_start(vout[1], vn1[:])

            ps0 = psa.tile([OWN, N], F32, tag="pslin")
            nc.tensor.matmul(ps0[:], smt["sela"][:], vstar[(0, "a")][:],
                             start=True, stop=False)
            nc.tensor.matmul(ps0[:], smt["selb"][:], vstar[(0, "b")][:],
                             start=False, stop=False)
            nc.tensor.matmul(ps0[:], smt["gphi"][:], phiow[:],
                             start=False, stop=False)
            nc.tensor.matmul(ps0[:], smt["gphih"][:], phih2[:],
                             start=False, stop=True)
            vn0 = spool.tile([OWN, N], F32, tag="vn0")
            nc.scalar.copy(vn0[:], ps0[:])
            nc.sync.dma_start(vout[0], vn0[:])

    nc.finalize()
    return nc


def kernel(V, P):
    global _PROG
    V = np.ascontiguousarray(V, np.float32)
    P = np.ascontiguousarray(P, np.float32)
    if _PROG is None:
        consts = _build_consts()
        nc = _build_program(consts)
        _PROG = (nc, consts)
    nc, consts = _PROG
    in_maps = []
    for c in range(NCORE):
        ch, xb = c // 4, c % 4
        x0 = OWN * xb
        rows = np.arange(x0 - PADL, x0 + OWN + PADR) % N
        m = {"vblk": np.ascontiguousarray(V[:, ch][:, rows, :]),
             "pblk": np.ascontiguousarray(P[ch][rows, :]),
             "pfull": np.ascontiguousarray(np.roll(P[ch], -x0, axis=0))}
        m.update(consts)
        in_maps.append(m)
    trace = os.environ.get("NSK_TRACE", "") == "1"
    res = run_bass_kernel_spmd(nc, in_maps, core_ids=list(range(NCORE)),
                               trace=trace)
    if trace:
        print(f"HW exec time: {res.exec_time_ns} ns")
        if res.instructions_and_trace:
            print("trace:", res.instructions_and_trace[1])
    V_new = np.empty((2, C, N, N), np.float32)
    P_new = np.empty((C, N, N), np.float32)
    for c in range(NCORE):
        ch, xb = c // 4, c % 4
        x0 = OWN * xb
        V_new[:, ch, x0:x0 + OWN, :] = res.results[c]["vout"]
        P_new[ch, x0:x0 + OWN, :] = res.results[c]["pout"]
    return V_new, P_new
